# revision 8
# baseline (speedup 1.0000x reference)
"""Trainium2 Bass kernel for the Conv-Adapter cross-attention block.

Contract: kernel(**inputs) takes the FULL unsharded inputs (as produced by
setup_inputs()) and returns the FULL [577, 64, 1024] float32 output.

Sharding: pure data-parallel over batch B=64 -> 8 batches per NeuronCore.

Per-batch dataflow on each core (all matmuls in float32r, 1 cycle/row):
  x[1:,b,:] -> LN (bn_stats, free-dim) -> PE-transpose -> XNT [dim, tok]
  text[:,b,:] -> LN -> affine -> kv_ext [nt, txt+2] (ones & rowsum cols)
  kv_ext -> PE-transpose -> KVT [txt, nt]
  QT_h   = W'_h^T @ XNT + r_h          (W' = diag(ln_g) Wq, r = ln_b @ Wq)
  logitsT= KVT^T @ QT; expT = exp(SCALE * logitsT)      [nt, tok]
  oT_h   = kv_ext^T @ expT  (+ softmax denom row s_h, kv-rowsum row)
  gates  = sigmoid(conv3x3(avg/max col)) via host-built 576x576 conv matrix
  x_se   = oT_0 * (g1/s1) + oT_1 * (g2/s2)              [txt, tok]
  y^T    = x_se^T @ conv_final^T  -> out[1+t, b, :]
"""

import sys

sys.path.insert(0, "/opt/trn_rl_repo")

import numpy as np

import concourse.bacc as bacc
import concourse.bass as bass
import concourse.mybir as mybir
import concourse.tile as tile
from concourse.bass_utils import run_bass_kernel_spmd
from concourse.masks import make_identity

TOK, B, DIM, TXT, NT, KS, H = 576, 64, 1024, 512, 1000, 3, 24
SCALE = TXT ** -0.5
NCORES = 8
BPC = B // NCORES  # batches per core
F32 = mybir.dt.float32
F32R = mybir.dt.float32r
EPS = 1e-5

AF = mybir.ActivationFunctionType
ALU = mybir.AluOpType

# token tiling: 576 = 4*128 + 64
TOK_TILES = [(i * 128, min(128, TOK - i * 128)) for i in range((TOK + 127) // 128)]
# NT tiling: 1000 = 7*128 + 104
NT_TILES = [(i * 128, min(128, NT - i * 128)) for i in range((NT + 127) // 128)]
NKO = len(NT_TILES)  # 8
DKO = DIM // 128     # 8
CKO = TXT // 128     # 4
NSL = [(0, 288), (288, 288)]  # tok free-dim split, both >=256 for f32r speed


def r32(ap):
    return ap.bitcast(F32R)


def build_nc():
    nc = bacc.Bacc(None, target_bir_lowering=False, debug=False)

    xs = nc.dram_tensor("xs", [TOK + 1, BPC, DIM], F32, kind="ExternalInput")
    ts = nc.dram_tensor("ts", [NT, BPC, TXT], F32, kind="ExternalInput")
    w1 = nc.dram_tensor("w1", [DIM, TXT], F32R, kind="ExternalInput")
    w2 = nc.dram_tensor("w2", [DIM, TXT], F32R, kind="ExternalInput")
    r1 = nc.dram_tensor("r1", [TXT], F32, kind="ExternalInput")
    r2 = nc.dram_tensor("r2", [TXT], F32, kind="ExternalInput")
    cft = nc.dram_tensor("cft", [TXT, DIM], F32R, kind="ExternalInput")
    m576 = nc.dram_tensor("m576", [TOK, TOK], mybir.dt.bfloat16, kind="ExternalInput")
    kvg = nc.dram_tensor("kvg", [1, TXT], F32, kind="ExternalInput")
    kvb = nc.dram_tensor("kvb", [1, TXT], F32, kind="ExternalInput")
    out = nc.dram_tensor("out", [TOK + 1, BPC, DIM], F32, kind="ExternalOutput")

    from contextlib import ExitStack
    with ExitStack() as ctx:
        tc = ctx.enter_context(tile.TileContext(nc))
        pp = ctx.enter_context(tc.tile_pool(name="persist", bufs=1))
        pxt = ctx.enter_context(tc.tile_pool(name="xt", bufs=2))
        pst = ctx.enter_context(tc.tile_pool(name="stats", bufs=4))
        pxnt = ctx.enter_context(tc.tile_pool(name="xnt", bufs=1))
        pkt = ctx.enter_context(tc.tile_pool(name="ktmp", bufs=2))
        pkvx = ctx.enter_context(tc.tile_pool(name="kvx", bufs=1))
        pkvt = ctx.enter_context(tc.tile_pool(name="kvt", bufs=1))
        pqt = ctx.enter_context(tc.tile_pool(name="qt", bufs=1))
        pet = ctx.enter_context(tc.tile_pool(name="et", bufs=1))
        pot = ctx.enter_context(tc.tile_pool(name="ot", bufs=1))
        pxse = ctx.enter_context(tc.tile_pool(name="xse", bufs=1))
        psm = ctx.enter_context(tc.tile_pool(name="sm", bufs=1))
        pyt = ctx.enter_context(tc.tile_pool(name="yt", bufs=2))
        ptmp = ctx.enter_context(tc.tile_pool(name="tmp", bufs=2))
        ptp = ctx.enter_context(tc.tile_pool(name="ptp", bufs=2, space="PSUM"))
        pmm = ctx.enter_context(tc.tile_pool(name="pmm", bufs=4, space="PSUM"))
        ppy = ctx.enter_context(tc.tile_pool(name="ppy", bufs=2, space="PSUM"))
        if True:
            # ---- persistent weights ----
            ident = pp.tile([128, 128], F32)
            make_identity(nc, ident)
            identr = r32(ident)

            sbW = []
            for wsrc in (w1, w2):
                t = pp.tile([128, DKO, TXT], F32R, tag=f"W_{wsrc.name}")
                nc.sync.dma_start(t, wsrc.rearrange("(o p) e -> p o e", p=128))
                sbW.append(t)
            sbR = []
            for rsrc in (r1, r2):
                t = pp.tile([128, CKO], F32, tag=f"R_{rsrc.name}")
                nc.sync.dma_start(t, rsrc.rearrange("(o p) -> p o", p=128))
                sbR.append(t)
            sbCFT = pp.tile([128, CKO, DIM], F32R)
            nc.sync.dma_start(sbCFT, cft.rearrange("(o p) d -> p o d", p=128))
            sbM = pp.tile([128, 5, TOK], mybir.dt.bfloat16)
            nc.sync.dma_start(sbM[:, :4, :], m576[:512].rearrange("(o p) d -> p o d", p=128))
            nc.sync.dma_start(sbM[:64, 4, :], m576[512:TOK])
            nc.vector.memset(sbM[64:, 4, :], 0.0)
            sbKVG = pp.tile([128, TXT], F32, tag="kvg")
            nc.sync.dma_start(sbKVG, kvg[:].to_broadcast((128, TXT)))
            sbKVB = pp.tile([128, TXT], F32, tag="kvb")
            nc.sync.dma_start(sbKVB, kvb[:].to_broadcast((128, TXT)))
            epsT = pp.tile([128, 1], F32, tag="eps")
            nc.vector.memset(epsT, EPS)
            zeros576 = pp.tile([128, TOK], F32, tag="zeros576")
            nc.vector.memset(zeros576, 0.0)
            ones8 = pp.tile([128, NKO], F32, tag="ones8")
            nc.vector.memset(ones8, 1.0)

            # x_cls passthrough
            nc.sync.dma_start(out[0:1, :, :], xs[0:1, :, :])

            for bi in range(BPC):
                # ============ x side: LN + transpose ============
                XNT = pxnt.tile([128, DKO, TOK], F32R, tag="XNT")
                for ti, (t0, tsz) in enumerate(TOK_TILES):
                    xt = pxt.tile([128, DIM], F32, tag="xt")
                    nc.sync.dma_start(xt[:tsz], xs[1 + t0:1 + t0 + tsz, bi, :])
                    st = pst.tile([128, 2, 6], F32, tag="xstat")
                    for g in range(2):
                        nc.vector.bn_stats(st[:tsz, g], xt[:tsz, g * 512:(g + 1) * 512])
                    mv = pst.tile([128, 2], F32, tag="xmv")
                    nc.vector.bn_aggr(mv[:tsz], st[:tsz])
                    rstd = pst.tile([128, 1], F32, tag="xrstd")
                    nc.scalar.activation(rstd[:tsz], mv[:tsz, 1:2], AF.Sqrt,
                                         bias=epsT[:tsz], scale=1.0)
                    nc.vector.reciprocal(rstd[:tsz], rstd[:tsz])
                    nmr = pst.tile([128, 1], F32, tag="xnmr")
                    nc.vector.scalar_tensor_tensor(nmr[:tsz], mv[:tsz, 0:1], -1.0,
                                                   rstd[:tsz], ALU.mult, ALU.mult)
                    # normalize in place on ACT
                    nc.scalar.activation(xt[:tsz], xt[:tsz], AF.Identity,
                                         bias=nmr[:tsz], scale=rstd[:tsz])
                    for ko in range(DKO):
                        pt = ptp.tile([128, 128], F32, tag="tp")
                        nc.tensor.transpose(pt[:, :tsz], xt[:tsz, ko * 128:(ko + 1) * 128],
                                            ident[:tsz, :tsz])
                        nc.vector.tensor_copy(XNT[:, ko, t0:t0 + tsz], pt[:, :tsz])

                # ============ kv side: LN + affine + extras ============
                # cols: 0..511 kv, 512 ones, 513 kv-rowsum
                kvx = pkvx.tile([128, NKO, TXT + 2], F32R, tag="kvx")
                nc.vector.tensor_copy(kvx[96:, NKO - 1, :], zeros576[96:, :TXT + 2])
                for ni, (n0, nsz) in enumerate(NT_TILES):
                    kt = pkt.tile([128, TXT], F32, tag="ktmp")
                    nc.sync.dma_start(kt[:nsz], ts[n0:n0 + nsz, bi, :])
                    st = pst.tile([128, 6], F32, tag="kstat")
                    nc.vector.bn_stats(st[:nsz], kt[:nsz])
                    mv = pst.tile([128, 2], F32, tag="kmv")
                    nc.vector.bn_aggr(mv[:nsz], st[:nsz])
                    rstd = pst.tile([128, 1], F32, tag="krstd")
                    nc.scalar.activation(rstd[:nsz], mv[:nsz, 1:2], AF.Sqrt,
                                         bias=epsT[:nsz], scale=1.0)
                    nc.vector.reciprocal(rstd[:nsz], rstd[:nsz])
                    nmr = pst.tile([128, 1], F32, tag="knmr")
                    nc.vector.scalar_tensor_tensor(nmr[:nsz], mv[:nsz, 0:1], -1.0,
                                                   rstd[:nsz], ALU.mult, ALU.mult)
                    nc.scalar.activation(kvx[:nsz, ni, :TXT], kt[:nsz], AF.Identity,
                                         bias=nmr[:nsz], scale=rstd[:nsz])
                    # affine: * g + b  (g, b broadcast rows)
                    nc.vector.tensor_tensor(kvx[:nsz, ni, :TXT], kvx[:nsz, ni, :TXT],
                                            sbKVG[:nsz], ALU.mult)
                    nc.vector.tensor_tensor(kvx[:nsz, ni, :TXT], kvx[:nsz, ni, :TXT],
                                            sbKVB[:nsz], ALU.add)
                nc.vector.tensor_copy(kvx[:, :, TXT:TXT + 1], ones8[:, :, None])
                kvs = pst.tile([128, NKO], F32, tag="kvs")
                nc.vector.reduce_sum(kvs[:, :, None], kvx[:, :, :TXT].bitcast(F32),
                                     axis=mybir.AxisListType.X)
                nc.vector.tensor_copy(kvx[:, :, TXT + 1:TXT + 2], kvs[:, :, None])

                # ============ KVT = transpose(kv) ============
                KVT = pkvt.tile([128, CKO, NT], F32R, tag="KVT")
                for ci in range(CKO):
                    for ni, (n0, nsz) in enumerate(NT_TILES):
                        pt = ptp.tile([128, 128], F32, tag="tp")
                        nc.tensor.transpose(pt, kvx[:, ni, ci * 128:(ci + 1) * 128].bitcast(F32),
                                            ident)
                        nc.vector.tensor_copy(KVT[:, ci, n0:n0 + nsz], pt[:, :nsz])

                oTs = []
                SRh = []
                for h in range(2):
                    # ============ QT_h = W'^T @ xn^T + r ============
                    QT = pqt.tile([128, CKO, TOK], F32R, tag="QT")
                    for co in range(CKO):
                        for s0, ssz in NSL:
                            pq = pmm.tile([128, 288], F32, tag="mm")
                            for ko in range(DKO):
                                nc.tensor.matmul(
                                    pq[:, :ssz],
                                    sbW[h][:, ko, co * 128:(co + 1) * 128],
                                    XNT[:, ko, s0:s0 + ssz],
                                    start=(ko == 0), stop=(ko == DKO - 1))
                            nc.scalar.activation(QT[:, co, s0:s0 + ssz], pq[:, :ssz],
                                                 AF.Identity, bias=sbR[h][:, co:co + 1],
                                                 scale=1.0)

                    # ============ expT = exp(SCALE * KVT^T @ QT) ============
                    ET = pet.tile([128, NKO, TOK], F32R, tag="ET")
                    nc.vector.tensor_copy(ET[96:, NKO - 1, :], zeros576[96:, :TOK])
                    for mi, (m0, msz) in enumerate(NT_TILES):
                        for s0, ssz in NSL:
                            pl = pmm.tile([128, 288], F32, tag="mm")
                            for co in range(CKO):
                                nc.tensor.matmul(
                                    pl[:msz, :ssz],
                                    KVT[:, co, m0:m0 + msz],
                                    QT[:, co, s0:s0 + ssz],
                                    start=(co == 0), stop=(co == CKO - 1))
                            nc.scalar.activation(ET[:msz, mi, s0:s0 + ssz],
                                                 pl[:msz, :ssz], AF.Exp, scale=SCALE)

                    # ============ oT_h = kv^T @ expT  (+ s, kvdot rows) ============
                    oT = pot.tile([128, CKO, TOK], F32, tag=f"oT{h}")
                    for co in range(CKO):
                        for s0, ssz in NSL:
                            po = pmm.tile([128, 288], F32, tag="mm")
                            for ko in range(NKO):
                                nc.tensor.matmul(
                                    po[:, :ssz],
                                    kvx[:, ko, co * 128:(co + 1) * 128],
                                    ET[:, ko, s0:s0 + ssz],
                                    start=(ko == 0), stop=(ko == NKO - 1))
                            nc.vector.tensor_copy(oT[:, co, s0:s0 + ssz], po[:, :ssz])
                    # extras: lhsT [ones, kvsum] -> rows (s_h@0, kvdot_h@1)
                    SR = psm.tile([2, TOK], F32, tag=f"SR{h}")
                    for s0, ssz in NSL:
                        pe = pmm.tile([128, 288], F32, tag="mm")
                        for ko in range(NKO):
                            nc.tensor.matmul(
                                pe[:2, :ssz],
                                kvx[:, ko, TXT:TXT + 2],
                                ET[:, ko, s0:s0 + ssz],
                                start=(ko == 0), stop=(ko == NKO - 1))
                        nc.vector.tensor_copy(SR[:, s0:s0 + ssz], pe[:2, :ssz])
                    SRh.append(SR)
                    oTs.append(oT)

                # ============ softmax denominators / gate inputs ============
                # SRh[h] rows: s@0, kvdot@1
                RSh = []
                for h in range(2):
                    RS = psm.tile([2, TOK], F32, tag=f"RS{h}")
                    nc.vector.reciprocal(RS, SRh[h])
                    RSh.append(RS)
                # columns: kvdot1, 1/s1, 1/s2
                SCOL = psm.tile([128, 5, 2], F32, tag="SCOL")
                RCOL = psm.tile([128, 5, 2], F32, tag="RCOL")
                RCOLb = psm.tile([128, 5, 2], F32, tag="RCOLb")
                for ti, (t0, tsz) in enumerate(TOK_TILES):
                    pt = ptp.tile([128, 128], F32, tag="tp")
                    nc.tensor.transpose(pt[:tsz, :2], SRh[0][:, t0:t0 + tsz], ident[:2, :2])
                    nc.vector.tensor_copy(SCOL[:tsz, ti], pt[:tsz, :2])
                    pt2 = ptp.tile([128, 128], F32, tag="tp")
                    nc.tensor.transpose(pt2[:tsz, :2], RSh[0][:, t0:t0 + tsz], ident[:2, :2])
                    nc.vector.tensor_copy(RCOL[:tsz, ti], pt2[:tsz, :2])
                    pt3 = ptp.tile([128, 128], F32, tag="tp")
                    nc.tensor.transpose(pt3[:tsz, :2], RSh[1][:, t0:t0 + tsz], ident[:2, :2])
                    nc.vector.tensor_copy(RCOLb[:tsz, ti], pt3[:tsz, :2])

                # channel max of raw oT_1 via PE transpose + free-dim reduce
                rm = psm.tile([128, 5, CKO], F32, tag="rm")
                for ti, (t0, tsz) in enumerate(TOK_TILES):
                    for co in range(CKO):
                        pt = ptp.tile([128, 128], F32, tag="tp")
                        nc.tensor.transpose(pt[:tsz, :], oTs[1][:, co, t0:t0 + tsz],
                                            ident)
                        nc.vector.reduce_max(rm[:tsz, ti, co:co + 1], pt[:tsz, :],
                                             axis=mybir.AxisListType.X)
                MXC = psm.tile([128, 5], F32, tag="MXC")
                nc.vector.reduce_max(MXC, rm, axis=mybir.AxisListType.X)

                # Z cols: [avg, max]; avg = kvdot1/s1/512, max = mx/s2
                Z = psm.tile([128, 5, 2], mybir.dt.bfloat16, tag="Z")
                nc.vector.scalar_tensor_tensor(Z[:, :, 0:1], SCOL[:, :, 1:2], 1.0 / TXT,
                                               RCOL[:, :, 0:1], ALU.mult, ALU.mult)
                nc.vector.scalar_tensor_tensor(Z[:, :, 1:2], MXC[:, :, None], 1.0,
                                               RCOLb[:, :, 0:1], ALU.mult, ALU.mult)
                nc.vector.memset(Z[64:, 4, :], 0.0)

                # gates per head: G_h = sigmoid(Z[:,:,h]^T @ M576T), F_h = G_h/s_h
                FBs = []
                for h in range(2):
                    G = psm.tile([1, TOK], F32, tag=f"G{h}")
                    for s0, ssz in NSL:
                        pg = pmm.tile([128, 288], F32, tag="mm")
                        for ki in range(5):
                            nc.tensor.matmul(pg[:1, :ssz], Z[:, ki, h:h + 1],
                                             sbM[:, ki, s0:s0 + ssz],
                                             start=(ki == 0), stop=(ki == 4))
                        nc.scalar.activation(G[:, s0:s0 + ssz], pg[:1, :ssz], AF.Sigmoid)
                    nc.vector.tensor_tensor(G, G, RSh[h][0:1, :], ALU.mult)
                    FB = psm.tile([128, TOK], F32, tag=f"FB{h}")
                    nc.gpsimd.partition_broadcast(FB, G[0:1, :])
                    FBs.append(FB)

                # ============ x_se = oT0 * f1 + oT1 * f2 ============
                xse = pxse.tile([128, CKO, TOK], F32R, tag="xse")
                for co in range(CKO):
                    t1 = ptmp.tile([128, TOK], F32, tag="t1")
                    nc.vector.tensor_tensor(t1, oTs[0][:, co, :],
                                            FBs[0], ALU.mult)
                    t2 = ptmp.tile([128, TOK], F32, tag="t1")
                    nc.vector.tensor_tensor(t2, oTs[1][:, co, :],
                                            FBs[1], ALU.mult)
                    nc.vector.tensor_tensor(xse[:, co, :], t1, t2, ALU.add)

                # ============ y^T = x_se^T @ CFT ============
                for ti, (t0, tsz) in enumerate(TOK_TILES):
                    for nh in range(2):
                        py = ppy.tile([128, 512], F32, tag="py")
                        for co in range(CKO):
                            nc.tensor.matmul(
                                py[:tsz], xse[:, co, t0:t0 + tsz],
                                sbCFT[:, co, nh * 512:(nh + 1) * 512],
                                start=(co == 0), stop=(co == CKO - 1))
                        yt = pyt.tile([128, 512], F32, tag="yt")
                        nc.scalar.copy(yt[:tsz], py[:tsz])
                        nc.sync.dma_start(
                            out[1 + t0:1 + t0 + tsz, bi, nh * 512:(nh + 1) * 512],
                            yt[:tsz])

    nc.finalize()
    return nc


_NC_CACHE = None


def _get_nc():
    global _NC_CACHE
    if _NC_CACHE is None:
        _NC_CACHE = build_nc()
    return _NC_CACHE


def make_in_maps(x, text_fea, ln_q1_g, ln_q1_b, ln_q2_g, ln_q2_b, ln_kv_g, ln_kv_b,
                 Wq1, Wq2, conv_dw, conv_final):
    x = np.asarray(x, np.float32)
    text_fea = np.asarray(text_fea, np.float32)
    w1 = np.ascontiguousarray(np.asarray(ln_q1_g, np.float32)[:, None] * np.asarray(Wq1, np.float32))
    w2 = np.ascontiguousarray(np.asarray(ln_q2_g, np.float32)[:, None] * np.asarray(Wq2, np.float32))
    r1 = np.asarray(ln_q1_b, np.float32) @ np.asarray(Wq1, np.float32)
    r2 = np.asarray(ln_q2_b, np.float32) @ np.asarray(Wq2, np.float32)
    cft = np.ascontiguousarray(np.asarray(conv_final, np.float32).T)

    # dwconv as a dense [out_spatial, in_spatial] matrix, fed transposed [i, o]
    w = np.asarray(conv_dw, np.float32).reshape(KS, KS)
    M = np.zeros((TOK, TOK), np.float32)
    for oy in range(H):
        for ox in range(H):
            o = oy * H + ox
            for dy in range(-1, 2):
                for dx in range(-1, 2):
                    iy, ix = oy + dy, ox + dx
                    if 0 <= iy < H and 0 <= ix < H:
                        M[o, iy * H + ix] = w[dy + 1, dx + 1]
    import ml_dtypes
    m576 = np.ascontiguousarray(M.T).astype(ml_dtypes.bfloat16)

    kvg = np.asarray(ln_kv_g, np.float32).reshape(1, TXT)
    kvb = np.asarray(ln_kv_b, np.float32).reshape(1, TXT)

    in_maps = []
    for c in range(NCORES):
        b0 = c * BPC
        in_maps.append({
            "xs": np.ascontiguousarray(x[:, b0:b0 + BPC, :]),
            "ts": np.ascontiguousarray(text_fea[:, b0:b0 + BPC, :]),
            "w1": w1, "w2": w2, "r1": r1, "r2": r2,
            "cft": cft, "m576": m576, "kvg": kvg, "kvb": kvb,
        })
    return in_maps


def kernel(**inputs):
    nc = _get_nc()
    in_maps = make_in_maps(**inputs)
    res = run_bass_kernel_spmd(nc, in_maps, core_ids=list(range(NCORES)))
    out = np.concatenate([res.results[c]["out"] for c in range(NCORES)], axis=1)
    return out
